# revision 1
# baseline (speedup 1.0000x reference)
"""Trainium2 Bass kernel for nn_Block_41077067219413.

Reference computation (B=2048, D=dim_in=4096, J=dim_out=4096):
    xf = x.astype(f32)                 # (B, D) in {0,1}
    mf = masks.astype(f32)             # (D, J) in {0,1}
    sums = xf @ mf + (1-xf) @ (1-mf)   # XNOR popcount over D
    out  = sums > thresholds[None, :]  # (B, J) bool

Identity: with x' = 2x-1 and m' = 2m-1 (both in {-1,+1}),
    A'[j,b] = sum_k m'[k,j] * x'[b,k] = 2*sums - D
    out     = A' > 2*th - D

Both operands are host-encoded as fp8e4 bytes (+1 = 0x38, -1 = 0xB8) and
host-tiled so every DMA row is 2048 contiguous bytes.  The device runs
one fp8 DoubleRow GEMM per core with no transposes, rowsum, or
threshold folding.  Work is sharded 2 (batch) x 4 (dim_out): each core
computes out_shard [1024 j, 1024 b] = m'^T @ x' with j on PSUM
partitions, so the threshold compare is a per-partition-scalar is_gt
(vector engine) or Sign activation (scalar engine), split across both.

Schedule: 8 k-split phases of 32 DR matmuls.  For each j-half, the two
batch-half accumulations run k-lower-half first (phases A,B), then
k-upper-half (C,D) followed by epilogues; this keeps all 8 PSUM banks
accumulating while first-touch DMA demand stays flat (~2MB per 6.9us
phase at the start, less later), below the measured ~280GB/s DMA
subsystem capacity, so the matmul stream starts at ~9.5us and never
stalls long enough to re-throttle the PE clock (HAM).  Warm-up matmuls
on a zeroed tile un-throttle the clock before data lands.  Input tiles
are enqueued in consumption order across the three DMA queues
(scalar/sync HWDGE early-critical, gpsimd SWDGE ramps up later), few
enough that DMA-semaphore reuse never targets an in-flight transfer.
"""

import numpy as np

B, D, J = 2048, 4096, 4096
NCORES = 8
BS, JS = 2, 4             # batch-shards x j-shards
BL = B // BS              # 1024 batch rows per core
JL = J // JS              # 1024 output cols per core
P = 128
KP = D // 256             # 16 k-pair steps (256 contraction each)
KQ = KP // 2              # 8 dram row-blocks (2 k-pairs = 2048B rows)
JT = JL // P              # 8 j-tiles of 128
BC = 512                  # batch free-dim chunk (one PSUM bank)
NBC = BL // BC            # 2 batch chunks
JH = JT // 2              # 4 j-tiles per phase
NWARM = 46

_cache = {}


def _build():
    import concourse.bacc as bacc
    import concourse.mybir as mybir
    import concourse.tile as tile

    dt = mybir.dt
    f8 = dt.float8e4
    f32 = dt.float32
    ALU = mybir.AluOpType
    AF = mybir.ActivationFunctionType
    DR = mybir.MatmulPerfMode.DoubleRow

    nc = bacc.Bacc("TRN2", target_bir_lowering=False, debug=False,
                   num_devices=NCORES)

    # host-tiled fp8 bytes; row r = (chunk*KQ + kq)*128 + ki holds 2048
    # contiguous bytes [kpp=0: ko0 512 | ko1 512 | kpp=1: ko0 | ko1]
    x_d = nc.dram_tensor("xp", [NBC * KQ * P, 2048], dt.uint8,
                         kind="ExternalInput")
    m_d = nc.dram_tensor("mp", [2 * KQ * P, 2048], dt.uint8,
                         kind="ExternalInput")
    c_d = nc.dram_tensor("cth", [P, JT], f32, kind="ExternalInput")
    # -(c+1) for the Sign-based epilogue on the Activation engine
    cn_d = nc.dram_tensor("cng", [P, JT], f32, kind="ExternalInput")
    o_d = nc.dram_tensor("out", [JL, BL], dt.uint8, kind="ExternalOutput")

    with tile.TileContext(nc) as tc:
        with (
            tc.tile_pool(name="const", bufs=1) as constp,
            tc.tile_pool(name="mk", bufs=1) as mkp,
            tc.tile_pool(name="xk", bufs=1) as xkp,
            tc.tile_pool(name="ob", bufs=1) as obp,
        ):
            # warm-up source: small zeroed tile, no DMA dependency
            wz = constp.tile([P, 2, 64], dt.uint8)
            nc.vector.memset(wz[:], 0)

            def hk_src(t_d, kp):
                return t_d[0:P, kp * 1024:(kp + 1) * 1024].rearrange(
                    "p (ko c) -> p ko c", ko=2)

            def kq_src(t_d, kq):
                return t_d[kq * P:(kq + 1) * P, :].rearrange(
                    "p (kpp ko c) -> p kpp ko c", kpp=2, ko=2)

            def b2_src(t_d, i):
                r0 = (KQ + 2 * i) * P
                return t_d[r0:r0 + 2 * P, :].rearrange(
                    "(kq p) (kpp ko c) -> p kq kpp ko c", p=P, kpp=2, ko=2)

            hk = {}   # (op, kp) -> [P, 2, BC]       op 0 = mask, 1 = x
            kqt = {}  # (op, kq) -> [P, 2, 2, BC]    chunk 0, kq 1..7
            b2 = {}   # (op, i)  -> [P, 2, 2, 2, BC] chunk 1, kq 2i..2i+1
            for op, pool in ((0, mkp), (1, xkp)):
                for kp in range(2):
                    hk[(op, kp)] = pool.tile([P, 2, BC], dt.uint8,
                                             name=f"hk{op}_{kp}")
                for kq in range(1, KQ):
                    kqt[(op, kq)] = pool.tile([P, 2, 2, BC], dt.uint8,
                                              name=f"kq{op}_{kq}")
                for i in range(4):
                    b2[(op, i)] = pool.tile([P, 2, 2, 2, BC], dt.uint8,
                                            name=f"b2_{op}_{i}")

            # DMA plan in consumption order (A: jh0/bc0 k-low; B: bc1
            # k-low; C: jh0/bc0 k-high; D: bc1 k-high; E/G: jh1 halves)
            plan = [
                ('s', hk[(0, 0)], hk_src(m_d, 0)),        # A
                ('y', hk[(1, 0)], hk_src(x_d, 0)),
                ('s', hk[(0, 1)], hk_src(m_d, 1)),
                ('y', hk[(1, 1)], hk_src(x_d, 1)),
                ('g', kqt[(0, 1)], kq_src(m_d, 1)),
                ('s', kqt[(1, 1)], kq_src(x_d, 1)),
                ('y', kqt[(0, 2)], kq_src(m_d, 2)),
                ('g', kqt[(1, 2)], kq_src(x_d, 2)),
                ('s', kqt[(0, 3)], kq_src(m_d, 3)),
                ('y', kqt[(1, 3)], kq_src(x_d, 3)),
                ('g', b2[(1, 0)], b2_src(x_d, 0)),        # B
                ('s', b2[(1, 1)], b2_src(x_d, 1)),
                ('y', kqt[(0, 4)], kq_src(m_d, 4)),       # C
                ('g', kqt[(1, 4)], kq_src(x_d, 4)),
                ('s', kqt[(0, 5)], kq_src(m_d, 5)),
                ('y', kqt[(1, 5)], kq_src(x_d, 5)),
                ('g', kqt[(0, 6)], kq_src(m_d, 6)),
                ('s', kqt[(1, 6)], kq_src(x_d, 6)),
                ('y', kqt[(0, 7)], kq_src(m_d, 7)),
                ('g', kqt[(1, 7)], kq_src(x_d, 7)),
                ('s', b2[(1, 2)], b2_src(x_d, 2)),        # D
                ('y', b2[(1, 3)], b2_src(x_d, 3)),
                ('s', 'cth', None),
                ('s', 'cng', None),
                ('g', b2[(0, 0)], b2_src(m_d, 0)),        # E
                ('s', b2[(0, 1)], b2_src(m_d, 1)),
                ('y', b2[(0, 2)], b2_src(m_d, 2)),        # G
                ('g', b2[(0, 3)], b2_src(m_d, 3)),
            ]
            qmap = {'g': nc.gpsimd, 's': nc.scalar, 'y': nc.sync}
            cth = constp.tile([P, JT], f32)
            cng = constp.tile([P, JT], f32)
            for q, dst, src in plan:
                if dst == 'cth':
                    nc.scalar.dma_start(cth[:], c_d[:])
                elif dst == 'cng':
                    nc.scalar.dma_start(cng[:], cn_d[:])
                else:
                    qmap[q].dma_start(dst[:], src)

            def mm_lhsT(jh, kp, j4):
                jsl = slice(j4 * P, (j4 + 1) * P)
                if jh == 0:
                    if kp < 2:
                        return hk[(0, kp)][:, :, jsl]
                    return kqt[(0, kp // 2)][:, kp % 2, :, jsl]
                kq = kp // 2
                return b2[(0, kq // 2)][:, kq % 2, kp % 2, :, jsl]

            def mm_rhs(bc, kp):
                if bc == 0:
                    if kp < 2:
                        return hk[(1, kp)][:]
                    return kqt[(1, kp // 2)][:, kp % 2, :, :]
                kq = kp // 2
                return b2[(1, kq // 2)][:, kq % 2, kp % 2, :, :]

            # fused output tiles: one per jt-pair [P, 2, BL]
            obs = [obp.tile([P, 2, BL], dt.uint8, name=f"ob{jp}")
                   for jp in range(JT // 2)]

            with tc.tile_pool(name="psacc", bufs=1, space="PSUM") as psacc:
                # PE warm-up: un-throttle HAM while first tiles land
                wps = psacc.tile([P, BC], f32, name="warm", tag="accw")
                for i in range(NWARM):
                    nc.tensor.matmul(
                        wps[0:32, 0:64], wz[:, :, 0:32].bitcast(f8),
                        wz[:].bitcast(f8),
                        start=(i == 0), stop=(i == NWARM - 1), perf_mode=DR)

                oqs = {0: nc.sync, 1: nc.scalar, 2: nc.scalar, 3: nc.sync}
                for jh in range(2):
                    # warm-up bank is reused as the first bc1 bank
                    ps = {}
                    for bc in range(NBC):
                        for j4 in range(JH):
                            tag = ("accw" if (bc, j4) == (1, 0)
                                   else f"acc{bc}_{j4}")
                            ps[(bc, j4)] = psacc.tile(
                                [P, BC], f32, name=f"acc{jh}_{bc}_{j4}",
                                tag=tag)
                    # phases: (bc0, k-low), (bc1, k-low), (bc0, k-high)+ep,
                    # (bc1, k-high)+ep
                    for bc, klo in ((0, 0), (1, 0), (0, 1), (1, 1)):
                        for kp in range(klo * KP // 2, (klo + 1) * KP // 2):
                            for j4 in range(JH):
                                nc.tensor.matmul(
                                    ps[(bc, j4)][:],
                                    mm_lhsT(jh, kp, j4).bitcast(f8),
                                    mm_rhs(bc, kp).bitcast(f8),
                                    start=(kp == 0), stop=(kp == KP - 1),
                                    perf_mode=DR)
                        if klo == 0:
                            continue
                        for j4 in range(JH):
                            jt = jh * JH + j4
                            osl = obs[jt // 2][:, jt % 2,
                                               bc * BC:(bc + 1) * BC]
                            if j4 % 2:
                                # A' and c are both even, so A'-(c+1) is
                                # odd: Sign never sees 0 and the strict
                                # compare is exact; the uint8 cast
                                # saturates -1 to 0.
                                nc.scalar.activation(
                                    osl, ps[(bc, j4)][:], AF.Sign,
                                    bias=cng[:, jt:jt + 1], scale=1.0)
                            else:
                                nc.vector.tensor_scalar(
                                    osl, ps[(bc, j4)][:],
                                    cth[:, jt:jt + 1], None, op0=ALU.is_gt)
                            if bc == NBC - 1 and j4 % 2:
                                jp = jt // 2
                                dst = o_d[jp * 2 * P:(jp + 1) * 2 * P,
                                          :].rearrange(
                                    "(j2 p) b -> p j2 b", p=P)
                                oqs[jp].dma_start(dst, obs[jp][:])

    nc.compile()
    return nc


def _get_nc():
    if "nc" not in _cache:
        _cache["nc"] = _build()
    return _cache["nc"]


def _encode_pm1(a01):
    """{0,1} array -> fp8e4 bytes for {-1,+1} (0xB8 / 0x38)."""
    return np.where(a01, np.uint8(0x38), np.uint8(0xB8))


def _tile_k_major(shard):
    """[4096, 1024] byte array (k-major) -> [2*KQ*128, 2048]: row
    (chunk*KQ + kq)*128 + ki = 2048 contiguous bytes covering the two
    k-pairs' ko-interleaved halves of one 512-column chunk."""
    t = shard.reshape(KQ, 2, 2, P, 2, BC)        # [kq, kpp, ko, ki, ch, c]
    t = t.transpose(4, 0, 3, 1, 2, 5)            # [ch, kq, ki, kpp, ko, c]
    return np.ascontiguousarray(t.reshape(2 * KQ * P, 2048))


def run(x, masks, thresholds, trace=False):
    """Run the SPMD kernel on 8 cores. Returns (out_bool, BassKernelResults)."""
    from concourse.bass_utils import run_bass_kernel_spmd

    nc = _get_nc()
    xT8 = np.ascontiguousarray(_encode_pm1(x.T != 0))          # [D, B]
    m8 = _encode_pm1(np.asarray(masks))                        # [D, J]
    cth = (2.0 * thresholds.astype(np.float32) - float(D))     # [J]
    in_maps = []
    for c in range(NCORES):
        bh, jq = c // JS, c % JS
        in_maps.append({
            "xp": _tile_k_major(xT8[:, bh * BL:(bh + 1) * BL]),
            "mp": _tile_k_major(m8[:, jq * JL:(jq + 1) * JL]),
            "cth": np.ascontiguousarray(
                cth[jq * JL:(jq + 1) * JL].reshape(JT, P).T),
            "cng": np.ascontiguousarray(
                -(cth[jq * JL:(jq + 1) * JL] + 1.0).reshape(JT, P).T),
        })
    res = run_bass_kernel_spmd(nc, in_maps, core_ids=list(range(NCORES)),
                               trace=trace)
    out = np.empty((B, J), dtype=np.uint8)
    for c in range(NCORES):
        bh, jq = c // JS, c % JS
        out[bh * BL:(bh + 1) * BL, jq * JL:(jq + 1) * JL] = \
            res.results[c]["out"].T
    return out.view(np.bool_), res


def kernel(x, masks, thresholds):
    x = np.asarray(x)
    masks = np.asarray(masks)
    thresholds = np.asarray(thresholds)
    out, _ = run(x, masks, thresholds, trace=False)
    return out



# revision 2
# speedup vs baseline: 1.0414x; 1.0414x over previous
"""Trainium2 Bass kernel for nn_Block_41077067219413.

Math: out = (x'@m' > 2*th - D) with x', m' the ±1 encodings of the
binary inputs, computed as one fp8 DoubleRow GEMM per core (2x4 batch
x j sharding, j on PSUM partitions) with the threshold compare fused
into the epilogue (vector is_gt / scalar Sign).

Schedule, from trace evidence (per-core numbers):

- PE stream: 256 DR matmuls [K=256, N=512] pace at the 216ns/MM
  hardware floor (512 cyc @2.4GHz).  Both batch chunks share each
  ldweights inside one merged kp loop (8 PSUM banks = 2bc x 4j4,
  16-deep accumulation chains).
- DMA: SWDGE (gpsimd-triggered) coalesces 2048B-row tiles into 4KB
  packets (~160-210GB/s); HWDGE (sync/scalar) manages only ~40-100GB/s.
  So the x streams (148GB/s demand) ride gpsimd as kq-granular tiles in
  consumption order, the m stream (74GB/s) is split across both HW
  queues (fine 128KB tiles for kp<4, 256KB kq tiles after), and the
  j-half-1 m plus all outputs ride gpsimd after the x stream drains.
- Outputs are written to a [JL/2, 2048] dram layout (2048B rows -> 4KB
  packets; the host unscrambles) as one 256KB DMA per jt-pair.
- 50 tiny warm-up matmuls bridge the framework preamble to the first
  tile arrival (~10.3us) and raise the HAM clock.
- The last 4 kp of each j-half run bc0 then bc1 with per-bc epilogues
  so the final epilogues split across vector+scalar and the last
  output DMA trails the last matmul by only ~2.5us.
"""

import numpy as np

B, D, J = 2048, 4096, 4096
NCORES = 8
BS, JS = 2, 4             # batch-shards x j-shards
BL = B // BS              # 1024 batch rows per core
JL = J // JS              # 1024 output cols per core
P = 128
KP = D // 256             # 16 k-pair steps (256 contraction each)
KQ = KP // 2              # 8 dram row-blocks (2 k-pairs = 2048B rows)
JT = JL // P              # 8 j-tiles of 128
BC = 512                  # batch free-dim chunk (one PSUM bank)
NBC = BL // BC            # 2 batch chunks
JH = JT // 2              # 4 j-tiles per j-half
NFINE = 4                 # kp tiles with fine (128KB) DMA granularity
KSPLIT = 12               # kp index where merged loop ends, bc-split tail
NWARM = 50

_cache = {}


def _build():
    import concourse.bacc as bacc
    import concourse.mybir as mybir
    import concourse.tile as tile

    dt = mybir.dt
    f8 = dt.float8e4
    f32 = dt.float32
    ALU = mybir.AluOpType
    AF = mybir.ActivationFunctionType
    DR = mybir.MatmulPerfMode.DoubleRow

    nc = bacc.Bacc("TRN2", target_bir_lowering=False, debug=False,
                   num_devices=NCORES)

    # host-tiled fp8 bytes; row r = (chunk*KQ + kq)*128 + ki holds 2048
    # contiguous bytes [kpp=0: ko0 512 | ko1 512 | kpp=1: ko0 | ko1]
    x_d = nc.dram_tensor("xp", [NBC * KQ * P, 2048], dt.uint8,
                         kind="ExternalInput")
    m_d = nc.dram_tensor("mp", [2 * KQ * P, 2048], dt.uint8,
                         kind="ExternalInput")
    c_d = nc.dram_tensor("cth", [P, JT], f32, kind="ExternalInput")
    # -(c+1) for the Sign-based epilogue on the Activation engine
    cn_d = nc.dram_tensor("cng", [P, JT], f32, kind="ExternalInput")
    # output rows are 2048B ([jp*128+p, j2*1024+b]) so the SWDGE can use
    # 4KB packets; the host unscrambles (cheap reshape/transpose).
    o_d = nc.dram_tensor("out", [JL // 2, 2 * BL], dt.uint8,
                         kind="ExternalOutput")

    with tile.TileContext(nc) as tc:
        with (
            tc.tile_pool(name="const", bufs=1) as constp,
            tc.tile_pool(name="mk", bufs=1) as mkp,
            tc.tile_pool(name="xk", bufs=1) as xkp,
            tc.tile_pool(name="ob", bufs=1) as obp,
        ):
            # warm-up source: small zeroed tile, no DMA dependency
            wz = constp.tile([P, 2, 64], dt.uint8)
            nc.vector.memset(wz[:], 0)

            def kp_src(t_d, chunk, kp):
                r0 = (chunk * KQ + kp // 2) * P
                return t_d[r0:r0 + P,
                           (kp % 2) * 1024:(kp % 2 + 1) * 1024].rearrange(
                    "p (ko c) -> p ko c", ko=2)

            def kq_src(t_d, chunk, kq):
                r0 = (chunk * KQ + kq) * P
                return t_d[r0:r0 + P, :].rearrange(
                    "p (kpp ko c) -> p kpp ko c", kpp=2, ko=2)

            def b2_src(t_d, chunk, i):
                r0 = (chunk * KQ + 2 * i) * P
                return t_d[r0:r0 + 2 * P, :].rearrange(
                    "(kq p) (kpp ko c) -> p kq kpp ko c", p=P, kpp=2, ko=2)

            # on-chip tiles.  Only gpsimd triggers the fast SWDGE queue
            # (4KB packets, ~160GB/s); sync/scalar HWDGE run ~40GB/s.
            # So: x streams (148GB/s demand) ride gpsimd with kq tiles;
            # the m stream (74GB/s) is fine-grained early and split
            # across both HW queues; m-jh1 + outputs ride gpsimd after
            # the x stream drains.
            m_kp = {kp: mkp.tile([P, 2, BC], dt.uint8, name=f"m_kp{kp}")
                    for kp in range(NFINE)}
            x_kp0 = {c: xkp.tile([P, 2, BC], dt.uint8, name=f"x{c}_kp0")
                     for c in range(2)}
            m_kq = {kq: mkp.tile([P, 2, 2, BC], dt.uint8, name=f"m_kq{kq}")
                    for kq in range(NFINE // 2, KQ)}
            x_kq = {(c, kq): xkp.tile([P, 2, 2, BC], dt.uint8,
                                      name=f"x{c}_kq{kq}")
                    for c in range(2) for kq in range(KQ)}
            m1b = {i: mkp.tile([P, 2, 2, 2, BC], dt.uint8, name=f"m1b{i}")
                   for i in range(4)}

            cth = constp.tile([P, JT], f32)
            cng = constp.tile([P, JT], f32)

            plan_sync = [(x_kp0[1], kp_src(x_d, 1, 0)),
                         (m_kp[1], kp_src(m_d, 0, 1)),
                         (m_kp[3], kp_src(m_d, 0, 3)),
                         (m_kq[3], kq_src(m_d, 0, 3)),
                         (m_kq[5], kq_src(m_d, 0, 5)),
                         (m_kq[7], kq_src(m_d, 0, 7))]
            plan_scalar = [(m_kp[0], kp_src(m_d, 0, 0)),
                           (cth, c_d[:]),
                           (cng, cn_d[:]),
                           (m_kp[2], kp_src(m_d, 0, 2)),
                           (m_kq[2], kq_src(m_d, 0, 2)),
                           (m_kq[4], kq_src(m_d, 0, 4)),
                           (m_kq[6], kq_src(m_d, 0, 6))]
            plan_gp = [(x_kp0[0], kp_src(x_d, 0, 0))]
            for kq in range(KQ):
                plan_gp.append((x_kq[(0, kq)], kq_src(x_d, 0, kq)))
                plan_gp.append((x_kq[(1, kq)], kq_src(x_d, 1, kq)))
            plan_gp += [(m1b[i], b2_src(m_d, 1, i)) for i in range(4)]
            for q, plan in ((nc.gpsimd, plan_gp), (nc.sync, plan_sync),
                            (nc.scalar, plan_scalar)):
                for dst, src in plan:
                    q.dma_start(dst[:], src)

            def m_sl(jh, kp, j4):
                jsl = slice(j4 * P, (j4 + 1) * P)
                if jh == 0:
                    if kp < NFINE:
                        return m_kp[kp][:, :, jsl]
                    return m_kq[kp // 2][:, kp % 2, :, jsl]
                kq = kp // 2
                return m1b[kq // 2][:, kq % 2, kp % 2, :, jsl]

            def x_t(c, kp):
                if kp == 0:
                    return x_kp0[c][:]
                return x_kq[(c, kp // 2)][:, kp % 2, :, :]

            # fused output tiles: one per jt-pair [P, 2, BL]
            obs = [obp.tile([P, 2, BL], dt.uint8, name=f"ob{jp}")
                   for jp in range(JT // 2)]

            with tc.tile_pool(name="psacc", bufs=1, space="PSUM") as psacc:
                # PE warm-up: raise the clock while first tiles land
                wps = psacc.tile([P, BC], f32, name="warm", tag="accw")
                for i in range(NWARM):
                    nc.tensor.matmul(
                        wps[0:32, 0:64], wz[:, :, 0:32].bitcast(f8),
                        wz[:].bitcast(f8),
                        start=(i == 0), stop=(i == NWARM - 1), perf_mode=DR)

                for jh in range(2):
                    ps = {}
                    for bc in range(NBC):
                        for j4 in range(JH):
                            tag = ("accw" if (bc, j4) == (1, 3)
                                   else f"acc{bc}_{j4}")
                            ps[(bc, j4)] = psacc.tile(
                                [P, BC], f32, name=f"acc{jh}_{bc}_{j4}",
                                tag=tag)
                    # merged loop: both bc per (kp, j4) share the ldweights
                    for kp in range(KSPLIT):
                        for j4 in range(JH):
                            msl = m_sl(jh, kp, j4).bitcast(f8)
                            for bc in range(NBC):
                                nc.tensor.matmul(
                                    ps[(bc, j4)][:], msl,
                                    x_t(bc, kp).bitcast(f8),
                                    start=(kp == 0), stop=False,
                                    perf_mode=DR)
                    # bc-split tail with per-bc epilogues
                    for bc in range(NBC):
                        for kp in range(KSPLIT, KP):
                            for j4 in range(JH):
                                nc.tensor.matmul(
                                    ps[(bc, j4)][:],
                                    m_sl(jh, kp, j4).bitcast(f8),
                                    x_t(bc, kp).bitcast(f8),
                                    start=False, stop=(kp == KP - 1),
                                    perf_mode=DR)
                        for j4 in range(JH):
                            jt = jh * JH + j4
                            osl = obs[jt // 2][:, jt % 2,
                                               bc * BC:(bc + 1) * BC]
                            # bc0 epilogues overlap the bc1 matmul tail on
                            # the vector engine; the final bc1 set splits
                            # across vector/scalar to halve the tail.
                            use_scalar = (bc == 1 and j4 % 2 == 1)
                            if use_scalar:
                                # A' and c are both even so A'-(c+1) is odd:
                                # Sign never sees 0; uint8 cast saturates
                                # -1 to 0.
                                nc.scalar.activation(
                                    osl, ps[(bc, j4)][:], AF.Sign,
                                    bias=cng[:, jt:jt + 1], scale=1.0)
                            else:
                                nc.vector.tensor_scalar(
                                    osl, ps[(bc, j4)][:],
                                    cth[:, jt:jt + 1], None, op0=ALU.is_gt)
                            if j4 % 2 == 1 and bc == NBC - 1:
                                # whole-jp 256KB out DMA, 2048B rows
                                jp = jt // 2
                                dst = o_d[jp * P:(jp + 1) * P, :].rearrange(
                                    "p (j2 b) -> p j2 b", j2=2)
                                nc.gpsimd.dma_start(dst, obs[jp][:])

    nc.compile()
    return nc


def _get_nc():
    if "nc" not in _cache:
        _cache["nc"] = _build()
    return _cache["nc"]


def _encode_pm1(a01):
    """{0,1} array -> fp8e4 bytes for {-1,+1} (0xB8 / 0x38)."""
    return np.where(a01, np.uint8(0x38), np.uint8(0xB8))


def _tile_k_major(shard):
    """[4096, 1024] byte array (k-major) -> [2*KQ*128, 2048]: row
    (chunk*KQ + kq)*128 + ki = 2048 contiguous bytes covering the two
    k-pairs' ko-interleaved halves of one 512-column chunk."""
    t = shard.reshape(KQ, 2, 2, P, 2, BC)        # [kq, kpp, ko, ki, ch, c]
    t = t.transpose(4, 0, 3, 1, 2, 5)            # [ch, kq, ki, kpp, ko, c]
    return np.ascontiguousarray(t.reshape(2 * KQ * P, 2048))


def run(x, masks, thresholds, trace=False):
    """Run the SPMD kernel on 8 cores. Returns (out_bool, BassKernelResults)."""
    from concourse.bass_utils import run_bass_kernel_spmd

    nc = _get_nc()
    xT8 = np.ascontiguousarray(_encode_pm1(x.T != 0))          # [D, B]
    m8 = _encode_pm1(np.asarray(masks))                        # [D, J]
    cth = (2.0 * thresholds.astype(np.float32) - float(D))     # [J]
    in_maps = []
    for c in range(NCORES):
        bh, jq = c // JS, c % JS
        in_maps.append({
            "xp": _tile_k_major(xT8[:, bh * BL:(bh + 1) * BL]),
            "mp": _tile_k_major(m8[:, jq * JL:(jq + 1) * JL]),
            "cth": np.ascontiguousarray(
                cth[jq * JL:(jq + 1) * JL].reshape(JT, P).T),
            "cng": np.ascontiguousarray(
                -(cth[jq * JL:(jq + 1) * JL] + 1.0).reshape(JT, P).T),
        })
    res = run_bass_kernel_spmd(nc, in_maps, core_ids=list(range(NCORES)),
                               trace=trace)
    out = np.empty((B, J), dtype=np.uint8)
    for c in range(NCORES):
        bh, jq = c // JS, c % JS
        # o_d rows are [jp*128+p, j2*1024+b]; j = jp*256 + j2*128 + p
        oc = res.results[c]["out"].reshape(JT // 2, P, 2, BL)
        oc = oc.transpose(0, 2, 1, 3).reshape(JL, BL)
        out[bh * BL:(bh + 1) * BL, jq * JL:(jq + 1) * JL] = oc.T
    return out.view(np.bool_), res


def kernel(x, masks, thresholds):
    x = np.asarray(x)
    masks = np.asarray(masks)
    thresholds = np.asarray(thresholds)
    out, _ = run(x, masks, thresholds, trace=False)
    return out


# revision 3
# speedup vs baseline: 1.0513x; 1.0095x over previous
"""Trainium2 Bass kernel for nn_Block_41077067219413.

Math: out = (x'@m' > 2*th - D) with x', m' the ±1 encodings of the
binary inputs, computed as one fp8 DoubleRow GEMM per core (2x4 batch
x j sharding, j on PSUM partitions) with the threshold compare fused
into the epilogue (vector is_gt / scalar Sign).

Schedule, from trace evidence (per-core numbers):

- PE stream: 256 DR matmuls [K=256, N=512] pace at the 216ns/MM
  hardware floor (512 cyc @2.4GHz).  Both batch chunks share each
  ldweights inside one merged kp loop (8 PSUM banks = 2bc x 4j4,
  16-deep accumulation chains).
- DMA: SWDGE (gpsimd-triggered) coalesces 2048B-row tiles into 4KB
  packets (~160-210GB/s); HWDGE (sync/scalar) manages only ~40-100GB/s.
  So the x streams (148GB/s demand) ride gpsimd as kq-granular tiles in
  consumption order, the m stream (74GB/s) is split across both HW
  queues (fine 128KB tiles for kp<4, 256KB kq tiles after), and the
  j-half-1 m plus all outputs ride gpsimd after the x stream drains.
- Outputs are written to a [JL/2, 2048] dram layout (2048B rows -> 4KB
  packets; the host unscrambles) as one 256KB DMA per jt-pair.
- 50 tiny warm-up matmuls bridge the framework preamble to the first
  tile arrival (~10.3us) and raise the HAM clock.
- The last 4 kp of each j-half run bc0 then bc1 with per-bc epilogues
  so the final epilogues split across vector+scalar and the last
  output DMA trails the last matmul by only ~2.5us.
"""

import numpy as np

B, D, J = 2048, 4096, 4096
NCORES = 8
BS, JS = 2, 4             # batch-shards x j-shards
BL = B // BS              # 1024 batch rows per core
JL = J // JS              # 1024 output cols per core
P = 128
KP = D // 256             # 16 k-pair steps (256 contraction each)
KQ = KP // 2              # 8 dram row-blocks (2 k-pairs = 2048B rows)
JT = JL // P              # 8 j-tiles of 128
BC = 512                  # batch free-dim chunk (one PSUM bank)
NBC = BL // BC            # 2 batch chunks
JH = JT // 2              # 4 j-tiles per j-half
NFINE = 4                 # kp tiles with fine (128KB) DMA granularity
KSPLIT = 12               # kp index where merged loop ends, bc-split tail
NWARM = 62

_cache = {}


def _build():
    import concourse.bacc as bacc
    import concourse.mybir as mybir
    import concourse.tile as tile

    dt = mybir.dt
    f8 = dt.float8e4
    f32 = dt.float32
    ALU = mybir.AluOpType
    AF = mybir.ActivationFunctionType
    DR = mybir.MatmulPerfMode.DoubleRow

    nc = bacc.Bacc("TRN2", target_bir_lowering=False, debug=False,
                   num_devices=NCORES)

    # host-tiled fp8 bytes; row r = (chunk*KQ + kq)*128 + ki holds 2048
    # contiguous bytes [kpp=0: ko0 512 | ko1 512 | kpp=1: ko0 | ko1]
    x_d = nc.dram_tensor("xp", [NBC * KQ * P, 2048], dt.uint8,
                         kind="ExternalInput")
    m_d = nc.dram_tensor("mp", [2 * KQ * P, 2048], dt.uint8,
                         kind="ExternalInput")
    c_d = nc.dram_tensor("cth", [P, JT], f32, kind="ExternalInput")
    # -(c+1) for the Sign-based epilogue on the Activation engine
    cn_d = nc.dram_tensor("cng", [P, JT], f32, kind="ExternalInput")
    # output rows are 2048B ([jp*128+p, j2*1024+b]) so the SWDGE can use
    # 4KB packets; the host unscrambles (cheap reshape/transpose).
    o_d = nc.dram_tensor("out", [JL // 2, 2 * BL], dt.uint8,
                         kind="ExternalOutput")

    with tile.TileContext(nc) as tc:
        with (
            tc.tile_pool(name="const", bufs=1) as constp,
            tc.tile_pool(name="mk", bufs=1) as mkp,
            tc.tile_pool(name="xk", bufs=1) as xkp,
            tc.tile_pool(name="ob", bufs=1) as obp,
        ):
            # warm-up source: small zeroed tile, no DMA dependency
            wz = constp.tile([P, 2, 64], dt.uint8)
            nc.vector.memset(wz[:], 0)

            def kp_src(t_d, chunk, kp):
                r0 = (chunk * KQ + kp // 2) * P
                return t_d[r0:r0 + P,
                           (kp % 2) * 1024:(kp % 2 + 1) * 1024].rearrange(
                    "p (ko c) -> p ko c", ko=2)

            def kq_src(t_d, chunk, kq):
                r0 = (chunk * KQ + kq) * P
                return t_d[r0:r0 + P, :].rearrange(
                    "p (kpp ko c) -> p kpp ko c", kpp=2, ko=2)

            def b2_src(t_d, chunk, i):
                r0 = (chunk * KQ + 2 * i) * P
                return t_d[r0:r0 + 2 * P, :].rearrange(
                    "(kq p) (kpp ko c) -> p kq kpp ko c", p=P, kpp=2, ko=2)

            # on-chip tiles, all kq-granular (256KB, 2048B rows).  Only
            # gpsimd triggers the fast SWDGE queue (4KB packets,
            # ~160-210GB/s); sync/scalar HWDGE move 2048B-row tiles at
            # ~85GB/s.  The three kq0 tiles ride one queue each in
            # parallel so the stream starts with kp0-3 supply already
            # pipelined; the m stream alternates between the HW queues,
            # x rides gpsimd, m-jh1 + outputs ride gpsimd after x.
            m_kq = {kq: mkp.tile([P, 2, 2, BC], dt.uint8, name=f"m_kq{kq}")
                    for kq in range(KQ)}
            x_kq = {(c, kq): xkp.tile([P, 2, 2, BC], dt.uint8,
                                      name=f"x{c}_kq{kq}")
                    for c in range(2) for kq in range(KQ)}
            m1b = {i: mkp.tile([P, 2, 2, 2, BC], dt.uint8, name=f"m1b{i}")
                   for i in range(4)}

            cth = constp.tile([P, JT], f32)
            cng = constp.tile([P, JT], f32)

            plan_sync = [(m_kq[0], kq_src(m_d, 0, 0)),
                         (m_kq[2], kq_src(m_d, 0, 2)),
                         (m_kq[4], kq_src(m_d, 0, 4)),
                         (m_kq[6], kq_src(m_d, 0, 6))]
            plan_scalar = [(x_kq[(0, 0)], kq_src(x_d, 0, 0)),
                           (m_kq[1], kq_src(m_d, 0, 1)),
                           (cth, c_d[:]),
                           (cng, cn_d[:]),
                           (m_kq[3], kq_src(m_d, 0, 3)),
                           (m_kq[5], kq_src(m_d, 0, 5)),
                           (m_kq[7], kq_src(m_d, 0, 7))]
            plan_gp = [(x_kq[(1, 0)], kq_src(x_d, 1, 0))]
            for kq in range(1, KQ):
                plan_gp.append((x_kq[(0, kq)], kq_src(x_d, 0, kq)))
                plan_gp.append((x_kq[(1, kq)], kq_src(x_d, 1, kq)))
            plan_gp += [(m1b[i], b2_src(m_d, 1, i)) for i in range(4)]
            for q, plan in ((nc.gpsimd, plan_gp), (nc.sync, plan_sync),
                            (nc.scalar, plan_scalar)):
                for dst, src in plan:
                    q.dma_start(dst[:], src)

            def m_sl(jh, kp, j4):
                jsl = slice(j4 * P, (j4 + 1) * P)
                if jh == 0:
                    return m_kq[kp // 2][:, kp % 2, :, jsl]
                kq = kp // 2
                return m1b[kq // 2][:, kq % 2, kp % 2, :, jsl]

            def x_t(c, kp):
                return x_kq[(c, kp // 2)][:, kp % 2, :, :]

            # fused output tiles: one per jt-pair [P, 2, BL]
            obs = [obp.tile([P, 2, BL], dt.uint8, name=f"ob{jp}")
                   for jp in range(JT // 2)]

            with tc.tile_pool(name="psacc", bufs=1, space="PSUM") as psacc:
                # PE warm-up: raise the clock while first tiles land
                wps = psacc.tile([P, BC], f32, name="warm", tag="accw")
                for i in range(NWARM):
                    nc.tensor.matmul(
                        wps[0:32, 0:64], wz[:, :, 0:32].bitcast(f8),
                        wz[:].bitcast(f8),
                        start=(i == 0), stop=(i == NWARM - 1), perf_mode=DR)

                for jh in range(2):
                    ps = {}
                    for bc in range(NBC):
                        for j4 in range(JH):
                            tag = ("accw" if (bc, j4) == (1, 3)
                                   else f"acc{bc}_{j4}")
                            ps[(bc, j4)] = psacc.tile(
                                [P, BC], f32, name=f"acc{jh}_{bc}_{j4}",
                                tag=tag)
                    # merged loop: both bc per (kp, j4) share the ldweights
                    for kp in range(KSPLIT):
                        for j4 in range(JH):
                            msl = m_sl(jh, kp, j4).bitcast(f8)
                            for bc in range(NBC):
                                nc.tensor.matmul(
                                    ps[(bc, j4)][:], msl,
                                    x_t(bc, kp).bitcast(f8),
                                    start=(kp == 0), stop=False,
                                    perf_mode=DR)
                    # bc-split tail with per-bc epilogues
                    for bc in range(NBC):
                        for kp in range(KSPLIT, KP):
                            for j4 in range(JH):
                                nc.tensor.matmul(
                                    ps[(bc, j4)][:],
                                    m_sl(jh, kp, j4).bitcast(f8),
                                    x_t(bc, kp).bitcast(f8),
                                    start=False, stop=(kp == KP - 1),
                                    perf_mode=DR)
                        for j4 in range(JH):
                            jt = jh * JH + j4
                            osl = obs[jt // 2][:, jt % 2,
                                               bc * BC:(bc + 1) * BC]
                            # bc0 epilogues overlap the bc1 matmul tail on
                            # the vector engine; the final bc1 set splits
                            # across vector/scalar to halve the tail.
                            use_scalar = (bc == 1 and j4 % 2 == 1)
                            if use_scalar:
                                # A' and c are both even so A'-(c+1) is odd:
                                # Sign never sees 0; uint8 cast saturates
                                # -1 to 0.
                                nc.scalar.activation(
                                    osl, ps[(bc, j4)][:], AF.Sign,
                                    bias=cng[:, jt:jt + 1], scale=1.0)
                            else:
                                nc.vector.tensor_scalar(
                                    osl, ps[(bc, j4)][:],
                                    cth[:, jt:jt + 1], None, op0=ALU.is_gt)
                            if j4 % 2 == 1 and bc == NBC - 1:
                                # whole-jp 256KB out DMA, 2048B rows
                                jp = jt // 2
                                dst = o_d[jp * P:(jp + 1) * P, :].rearrange(
                                    "p (j2 b) -> p j2 b", j2=2)
                                nc.gpsimd.dma_start(dst, obs[jp][:])

    nc.compile()
    return nc


def _get_nc():
    if "nc" not in _cache:
        _cache["nc"] = _build()
    return _cache["nc"]


def _encode_pm1(a01):
    """{0,1} array -> fp8e4 bytes for {-1,+1} (0xB8 / 0x38)."""
    return np.where(a01, np.uint8(0x38), np.uint8(0xB8))


def _tile_k_major(shard):
    """[4096, 1024] byte array (k-major) -> [2*KQ*128, 2048]: row
    (chunk*KQ + kq)*128 + ki = 2048 contiguous bytes covering the two
    k-pairs' ko-interleaved halves of one 512-column chunk."""
    t = shard.reshape(KQ, 2, 2, P, 2, BC)        # [kq, kpp, ko, ki, ch, c]
    t = t.transpose(4, 0, 3, 1, 2, 5)            # [ch, kq, ki, kpp, ko, c]
    return np.ascontiguousarray(t.reshape(2 * KQ * P, 2048))


def run(x, masks, thresholds, trace=False):
    """Run the SPMD kernel on 8 cores. Returns (out_bool, BassKernelResults)."""
    from concourse.bass_utils import run_bass_kernel_spmd

    nc = _get_nc()
    xT8 = np.ascontiguousarray(_encode_pm1(x.T != 0))          # [D, B]
    m8 = _encode_pm1(np.asarray(masks))                        # [D, J]
    cth = (2.0 * thresholds.astype(np.float32) - float(D))     # [J]
    in_maps = []
    for c in range(NCORES):
        bh, jq = c // JS, c % JS
        in_maps.append({
            "xp": _tile_k_major(xT8[:, bh * BL:(bh + 1) * BL]),
            "mp": _tile_k_major(m8[:, jq * JL:(jq + 1) * JL]),
            "cth": np.ascontiguousarray(
                cth[jq * JL:(jq + 1) * JL].reshape(JT, P).T),
            "cng": np.ascontiguousarray(
                -(cth[jq * JL:(jq + 1) * JL] + 1.0).reshape(JT, P).T),
        })
    res = run_bass_kernel_spmd(nc, in_maps, core_ids=list(range(NCORES)),
                               trace=trace)
    out = np.empty((B, J), dtype=np.uint8)
    for c in range(NCORES):
        bh, jq = c // JS, c % JS
        # o_d rows are [jp*128+p, j2*1024+b]; j = jp*256 + j2*128 + p
        oc = res.results[c]["out"].reshape(JT // 2, P, 2, BL)
        oc = oc.transpose(0, 2, 1, 3).reshape(JL, BL)
        out[bh * BL:(bh + 1) * BL, jq * JL:(jq + 1) * JL] = oc.T
    return out.view(np.bool_), res


def kernel(x, masks, thresholds):
    x = np.asarray(x)
    masks = np.asarray(masks)
    thresholds = np.asarray(thresholds)
    out, _ = run(x, masks, thresholds, trace=False)
    return out


# revision 4
# speedup vs baseline: 1.0604x; 1.0086x over previous
"""Trainium2 Bass kernel for nn_Block_41077067219413.

Math: out = (x'@m' > 2*th - D) with x', m' the ±1 encodings of the
binary inputs, computed as one fp8 DoubleRow GEMM per core (2x4 batch
x j sharding, j on PSUM partitions) with the threshold compare fused
into the epilogue (vector is_gt / scalar Sign).

Schedule, from trace evidence (per-core numbers):

- PE stream: 256 DR matmuls [K=256, N=512] pace at the 216ns/MM
  hardware floor (512 cyc @2.4GHz).  Both batch chunks share each
  ldweights inside one merged kp loop (8 PSUM banks = 2bc x 4j4,
  16-deep accumulation chains).
- DMA: SWDGE (gpsimd-triggered) coalesces 2048B-row tiles into 4KB
  packets (~160-210GB/s); HWDGE (sync/scalar) manages only ~40-100GB/s.
  So the x streams (148GB/s demand) ride gpsimd as kq-granular tiles in
  consumption order, the m stream (74GB/s) is split across both HW
  queues (fine 128KB tiles for kp<4, 256KB kq tiles after), and the
  j-half-1 m plus all outputs ride gpsimd after the x stream drains.
- Outputs are written to a [JL/2, 2048] dram layout (2048B rows -> 4KB
  packets; the host unscrambles) as one 256KB DMA per jt-pair.
- 50 tiny warm-up matmuls bridge the framework preamble to the first
  tile arrival (~10.3us) and raise the HAM clock.
- The last 4 kp of each j-half run bc0 then bc1 with per-bc epilogues
  so the final epilogues split across vector+scalar and the last
  output DMA trails the last matmul by only ~2.5us.
"""

import numpy as np

B, D, J = 2048, 4096, 4096
NCORES = 8
BS, JS = 2, 4             # batch-shards x j-shards
BL = B // BS              # 1024 batch rows per core
JL = J // JS              # 1024 output cols per core
P = 128
KP = D // 256             # 16 k-pair steps (256 contraction each)
KQ = KP // 2              # 8 dram row-blocks (2 k-pairs = 2048B rows)
JT = JL // P              # 8 j-tiles of 128
BC = 512                  # batch free-dim chunk (one PSUM bank)
NBC = BL // BC            # 2 batch chunks
JH = JT // 2              # 4 j-tiles per j-half
NFINE = 4                 # kp tiles with fine (128KB) DMA granularity
KSPLIT = 12               # kp index where merged loop ends, bc-split tail
NWARM = 62

_cache = {}


def _build():
    import concourse.bacc as bacc
    import concourse.mybir as mybir
    import concourse.tile as tile

    dt = mybir.dt
    f8 = dt.float8e4
    f32 = dt.float32
    ALU = mybir.AluOpType
    AF = mybir.ActivationFunctionType
    DR = mybir.MatmulPerfMode.DoubleRow

    nc = bacc.Bacc("TRN2", target_bir_lowering=False, debug=False,
                   num_devices=NCORES)

    # host-tiled fp8 bytes; row r = (chunk*KQ + kq)*128 + ki holds 2048
    # contiguous bytes [kpp=0: ko0 512 | ko1 512 | kpp=1: ko0 | ko1]
    x_d = nc.dram_tensor("xp", [NBC * KQ * P, 2048], dt.uint8,
                         kind="ExternalInput")
    m_d = nc.dram_tensor("mp", [2 * KQ * P, 2048], dt.uint8,
                         kind="ExternalInput")
    c_d = nc.dram_tensor("cth", [P, JT], f32, kind="ExternalInput")
    # -(c+1) for the Sign-based epilogue on the Activation engine
    cn_d = nc.dram_tensor("cng", [P, JT], f32, kind="ExternalInput")
    # output rows are 2048B ([jp*128+p, j2*1024+b]) so the SWDGE can use
    # 4KB packets; the host unscrambles (cheap reshape/transpose).
    o_d = nc.dram_tensor("out", [JL // 2, 2 * BL], dt.uint8,
                         kind="ExternalOutput")

    with tile.TileContext(nc) as tc:
        with (
            tc.tile_pool(name="const", bufs=1) as constp,
            tc.tile_pool(name="mk", bufs=1) as mkp,
            tc.tile_pool(name="xk", bufs=1) as xkp,
            tc.tile_pool(name="ob", bufs=1) as obp,
        ):
            # warm-up source: small zeroed tile, no DMA dependency
            wz = constp.tile([P, 2, 64], dt.uint8)
            nc.vector.memset(wz[:], 0)

            def kp_src(t_d, chunk, kp):
                r0 = (chunk * KQ + kp // 2) * P
                return t_d[r0:r0 + P,
                           (kp % 2) * 1024:(kp % 2 + 1) * 1024].rearrange(
                    "p (ko c) -> p ko c", ko=2)

            def kq_src(t_d, chunk, kq):
                r0 = (chunk * KQ + kq) * P
                return t_d[r0:r0 + P, :].rearrange(
                    "p (kpp ko c) -> p kpp ko c", kpp=2, ko=2)

            def b2_src(t_d, chunk, i):
                r0 = (chunk * KQ + 2 * i) * P
                return t_d[r0:r0 + 2 * P, :].rearrange(
                    "(kq p) (kpp ko c) -> p kq kpp ko c", p=P, kpp=2, ko=2)

            # on-chip tiles, all kq-granular (256KB, 2048B rows).  Only
            # gpsimd triggers the fast SWDGE queue (4KB packets,
            # ~160-210GB/s); sync/scalar HWDGE move 2048B-row tiles at
            # ~85GB/s.  The three kq0 tiles ride one queue each in
            # parallel so the stream starts with kp0-3 supply already
            # pipelined; the m stream alternates between the HW queues,
            # x rides gpsimd, m-jh1 + outputs ride gpsimd after x.
            m_kq = {kq: mkp.tile([P, 2, 2, BC], dt.uint8, name=f"m_kq{kq}")
                    for kq in range(KQ)}
            x_kq = {(c, kq): xkp.tile([P, 2, 2, BC], dt.uint8,
                                      name=f"x{c}_kq{kq}")
                    for c in range(2) for kq in range(KQ)}
            m1b = {i: mkp.tile([P, 2, 2, 2, BC], dt.uint8, name=f"m1b{i}")
                   for i in range(4)}

            cth = constp.tile([P, JT], f32)
            cng = constp.tile([P, JT], f32)

            plan_sync = [(m_kq[0], kq_src(m_d, 0, 0)),
                         (x_kq[(0, 1)], kq_src(x_d, 0, 1)),
                         (m_kq[2], kq_src(m_d, 0, 2)),
                         (m_kq[4], kq_src(m_d, 0, 4)),
                         (m_kq[6], kq_src(m_d, 0, 6))]
            plan_scalar = [(x_kq[(0, 0)], kq_src(x_d, 0, 0)),
                           (m_kq[1], kq_src(m_d, 0, 1)),
                           (cth, c_d[:]),
                           (cng, cn_d[:]),
                           (m_kq[3], kq_src(m_d, 0, 3)),
                           (m_kq[5], kq_src(m_d, 0, 5)),
                           (m_kq[7], kq_src(m_d, 0, 7))]
            plan_gp = [(x_kq[(1, 0)], kq_src(x_d, 1, 0)),
                       (x_kq[(1, 1)], kq_src(x_d, 1, 1))]
            for kq in range(2, KQ):
                plan_gp.append((x_kq[(0, kq)], kq_src(x_d, 0, kq)))
                plan_gp.append((x_kq[(1, kq)], kq_src(x_d, 1, kq)))
            plan_gp += [(m1b[i], b2_src(m_d, 1, i)) for i in range(4)]
            for q, plan in ((nc.gpsimd, plan_gp), (nc.sync, plan_sync),
                            (nc.scalar, plan_scalar)):
                for dst, src in plan:
                    q.dma_start(dst[:], src)

            def m_sl(jh, kp, j4):
                jsl = slice(j4 * P, (j4 + 1) * P)
                if jh == 0:
                    return m_kq[kp // 2][:, kp % 2, :, jsl]
                kq = kp // 2
                return m1b[kq // 2][:, kq % 2, kp % 2, :, jsl]

            def x_t(c, kp):
                return x_kq[(c, kp // 2)][:, kp % 2, :, :]

            # fused output tiles: one per jt-pair [P, 2, BL]
            obs = [obp.tile([P, 2, BL], dt.uint8, name=f"ob{jp}")
                   for jp in range(JT // 2)]

            with tc.tile_pool(name="psacc", bufs=1, space="PSUM") as psacc:
                # PE warm-up: raise the clock while first tiles land
                wps = psacc.tile([P, BC], f32, name="warm", tag="accw")
                for i in range(NWARM):
                    nc.tensor.matmul(
                        wps[0:32, 0:64], wz[:, :, 0:32].bitcast(f8),
                        wz[:].bitcast(f8),
                        start=(i == 0), stop=(i == NWARM - 1), perf_mode=DR)

                for jh in range(2):
                    ps = {}
                    for bc in range(NBC):
                        for j4 in range(JH):
                            tag = ("accw" if (bc, j4) == (1, 3)
                                   else f"acc{bc}_{j4}")
                            ps[(bc, j4)] = psacc.tile(
                                [P, BC], f32, name=f"acc{jh}_{bc}_{j4}",
                                tag=tag)
                    # merged loop: both bc per (kp, j4) share the ldweights
                    for kp in range(KSPLIT):
                        for j4 in range(JH):
                            msl = m_sl(jh, kp, j4).bitcast(f8)
                            for bc in range(NBC):
                                nc.tensor.matmul(
                                    ps[(bc, j4)][:], msl,
                                    x_t(bc, kp).bitcast(f8),
                                    start=(kp == 0), stop=False,
                                    perf_mode=DR)
                    # bc-split tail with per-bc epilogues
                    for bc in range(NBC):
                        for kp in range(KSPLIT, KP):
                            for j4 in range(JH):
                                nc.tensor.matmul(
                                    ps[(bc, j4)][:],
                                    m_sl(jh, kp, j4).bitcast(f8),
                                    x_t(bc, kp).bitcast(f8),
                                    start=False, stop=(kp == KP - 1),
                                    perf_mode=DR)
                        for j4 in range(JH):
                            jt = jh * JH + j4
                            osl = obs[jt // 2][:, jt % 2,
                                               bc * BC:(bc + 1) * BC]
                            # bc0 epilogues overlap the bc1 matmul tail on
                            # the vector engine; the final bc1 set splits
                            # across vector/scalar to halve the tail.
                            use_scalar = (bc == 1 and j4 % 2 == 1)
                            if use_scalar:
                                # A' and c are both even so A'-(c+1) is odd:
                                # Sign never sees 0; uint8 cast saturates
                                # -1 to 0.
                                nc.scalar.activation(
                                    osl, ps[(bc, j4)][:], AF.Sign,
                                    bias=cng[:, jt:jt + 1], scale=1.0)
                            else:
                                nc.vector.tensor_scalar(
                                    osl, ps[(bc, j4)][:],
                                    cth[:, jt:jt + 1], None, op0=ALU.is_gt)
                            if j4 % 2 == 1 and bc == NBC - 1:
                                # whole-jp 256KB out DMA, 2048B rows
                                jp = jt // 2
                                dst = o_d[jp * P:(jp + 1) * P, :].rearrange(
                                    "p (j2 b) -> p j2 b", j2=2)
                                nc.gpsimd.dma_start(dst, obs[jp][:])

    nc.compile()
    return nc


def _get_nc():
    if "nc" not in _cache:
        _cache["nc"] = _build()
    return _cache["nc"]


def _encode_pm1(a01):
    """{0,1} array -> fp8e4 bytes for {-1,+1} (0xB8 / 0x38)."""
    return np.where(a01, np.uint8(0x38), np.uint8(0xB8))


def _tile_k_major(shard):
    """[4096, 1024] byte array (k-major) -> [2*KQ*128, 2048]: row
    (chunk*KQ + kq)*128 + ki = 2048 contiguous bytes covering the two
    k-pairs' ko-interleaved halves of one 512-column chunk."""
    t = shard.reshape(KQ, 2, 2, P, 2, BC)        # [kq, kpp, ko, ki, ch, c]
    t = t.transpose(4, 0, 3, 1, 2, 5)            # [ch, kq, ki, kpp, ko, c]
    return np.ascontiguousarray(t.reshape(2 * KQ * P, 2048))


def run(x, masks, thresholds, trace=False):
    """Run the SPMD kernel on 8 cores. Returns (out_bool, BassKernelResults)."""
    from concourse.bass_utils import run_bass_kernel_spmd

    nc = _get_nc()
    xT8 = np.ascontiguousarray(_encode_pm1(x.T != 0))          # [D, B]
    m8 = _encode_pm1(np.asarray(masks))                        # [D, J]
    cth = (2.0 * thresholds.astype(np.float32) - float(D))     # [J]
    in_maps = []
    for c in range(NCORES):
        bh, jq = c // JS, c % JS
        in_maps.append({
            "xp": _tile_k_major(xT8[:, bh * BL:(bh + 1) * BL]),
            "mp": _tile_k_major(m8[:, jq * JL:(jq + 1) * JL]),
            "cth": np.ascontiguousarray(
                cth[jq * JL:(jq + 1) * JL].reshape(JT, P).T),
            "cng": np.ascontiguousarray(
                -(cth[jq * JL:(jq + 1) * JL] + 1.0).reshape(JT, P).T),
        })
    res = run_bass_kernel_spmd(nc, in_maps, core_ids=list(range(NCORES)),
                               trace=trace)
    out = np.empty((B, J), dtype=np.uint8)
    for c in range(NCORES):
        bh, jq = c // JS, c % JS
        # o_d rows are [jp*128+p, j2*1024+b]; j = jp*256 + j2*128 + p
        oc = res.results[c]["out"].reshape(JT // 2, P, 2, BL)
        oc = oc.transpose(0, 2, 1, 3).reshape(JL, BL)
        out[bh * BL:(bh + 1) * BL, jq * JL:(jq + 1) * JL] = oc.T
    return out.view(np.bool_), res


def kernel(x, masks, thresholds):
    x = np.asarray(x)
    masks = np.asarray(masks)
    thresholds = np.asarray(thresholds)
    out, _ = run(x, masks, thresholds, trace=False)
    return out
